# revision 3
# baseline (speedup 1.0000x reference)
"""Trainium2 Bass kernel for nn_CvxDifflayer (30x30 grid shortest-path LP via PDHG).

Strategy
--------
The LP matrix A is the (negated) oriented incidence matrix of a 30x30 grid with
8-neighbor connectivity: 900 "internal" edges (in->out of each cell, carrying
the weights as costs) plus 6844 neighbor edges.  A@x and A.T@y are therefore
8-neighbor *stencils* on the grid, not dense matmuls.

Device layout: edges live in a 90-partition x 96-column fp32 tile.  Partition
30*a+i holds grid row i of direction-block a (a = p+1, p in {-1,0,1}); column
32*b+j holds grid column j of direction-block b (b = q+1).  Chunk (a,b)=(1,1)
is the internal edge; the other 8 chunks are the neighbor directions.  All of
the PDHG linear algebra (A.T y stencil, A x_bar stencil with x_bar = 2*x_new -
x folded in, constants, and masking via a -2^100 additive "INV" term that the
[0,1] clip turns into exact zeros) runs as small PSUM-accumulating TensorE
matmuls; each iteration needs only two VectorE ops (the clip and the y-state
update).

The step size tau = 0.95/||A||_2 is computed on the host with the same fp32
power iteration the reference uses (the trajectory is insensitive to ulp-level
tau perturbations; validated numerically).  The ||x|| ridge term (MU = 1e-8)
affects fp32 results only while ||x|| is tiny, so the exact per-iteration norm
chain runs for the first PEEL iterations and s = 1 - tau*MU/||x|| is exactly
1.0f afterwards (validated bit-identical).

All 8 NeuronCores run the identical single-core program (the LP is one small
sequential solve; per the sharding hint replication beats tensor parallelism);
core 0's output is returned.
"""

import sys

import numpy as np

sys.path.insert(0, "/opt/trn_rl_repo")

import concourse.bacc as bacc
import concourse.mybir as mybir
import concourse.tile as tile
from concourse.bass_utils import run_bass_kernel_spmd

F32 = mybir.dt.float32
ALU = mybir.AluOpType
ACTF = mybir.ActivationFunctionType

XM = YM = 30
MU = 1e-8
N_ITERS = 1500
PEEL = 16            # iterations with the exact ||x|| ridge term
UNROLL = 28          # loop body unroll (even, preserves buffer parity)
BIG = float(2.0 ** 100)
N_CORES = 8
USE_BCAST_RHS = True  # stride-0 rhs AP for the u_out broadcast matmul


# ---------------------------------------------------------------- host side

def _host_prep(weights, A):
    """Decode the grid structure from A, run the fp32 power iteration for tau,
    and build every constant/weight tile the device program needs."""
    A = np.asarray(A, np.float32)
    w = np.asarray(weights, np.float32)
    V = A.shape[1]

    tails = np.argmax(A, 0)
    heads = np.argmin(A, 0)
    mask = np.zeros((3, 3, XM, YM), np.float32)
    for e in range(V):
        t, h = int(tails[e]), int(heads[e])
        i, j = divmod(t // 2, YM)
        if t % 2 == 0:
            mask[1, 1, i, j] = 1.0
        else:
            i2, j2 = divmod(h // 2, YM)
            mask[i2 - i + 1, j2 - j + 1, i, j] = 1.0

    # fp32 power iteration (same op structure as the reference)
    v = np.full(V, 1.0 / np.sqrt(np.float32(V)), np.float32)
    for _ in range(30):
        wv = A.T @ (A @ v)
        v = wv / np.float32(np.linalg.norm(wv))
    L = np.float32(np.linalg.norm(A @ v))
    tau = np.float32(0.95) / L
    st = np.float32(tau * tau)

    consts = {}

    # CONST tile: -BIG at invalid/pad lanes (clipped to exact 0), -tau*c at ch4
    const = np.zeros((90, 96), np.float32)
    for a in range(3):
        for b in range(3):
            blk = const[30 * a:30 * a + 30, 32 * b:32 * b + 32]
            blk[:, 30:] = -BIG
            blk[:, :30] = np.where(mask[a, b] > 0, 0.0, -BIG)
    const[30:60, 32:62] += -tau * w
    consts["CONST"] = const

    consts["W_I90"] = np.eye(90, dtype=np.float32)
    consts["W_ONES90"] = np.ones((90, 90), np.float32)

    # -u_out broadcast into all 9 chunks
    w_mout = np.zeros((30, 90), np.float32)
    for a in range(3):
        for i in range(30):
            w_mout[i, 30 * a + i] = -1.0
    consts["W_MOUT"] = w_mout

    # +u_in[i+p, j+q] shifts (partition direction handled by W, q by rhs offset)
    w_shift = np.zeros((30, 90), np.float32)
    for a in range(3):
        p = a - 1
        for i in range(30):
            if 0 <= i + p < 30:
                w_shift[i + p, 30 * a + i] = 1.0
    consts["W_SHIFT"] = w_shift

    # ch4 correction: uniform chunks gave -u_out + u_in; target +u_out - u_in
    w_fixo = np.zeros((30, 90), np.float32)
    w_fixi = np.zeros((30, 90), np.float32)
    for i in range(30):
        w_fixo[i, 30 + i] = 2.0
        w_fixi[i, 30 + i] = -2.0
    consts["W_FIXO"] = w_fixo
    consts["W_FIXI"] = w_fixi

    # Z weights: Z accumulates st*A@(2*x_new - x_old) - st*b
    zin_b02 = np.zeros((90, 30), np.float32)
    for a in range(3):
        p = a - 1
        for i2 in range(30):
            i = i2 - p
            if 0 <= i < 30:
                zin_b02[30 * a + i, i2] = -st
    zin_b1 = zin_b02.copy()
    zin_b1[30:60, :] *= -1.0

    zout_b02 = np.zeros((90, 30), np.float32)
    for a in range(3):
        for i in range(30):
            zout_b02[30 * a + i, i] = st
    zout_b1 = zout_b02.copy()
    zout_b1[30:60, :] *= -1.0

    consts["WZI2_N"] = 2.0 * zin_b02
    consts["WZI1_N"] = 2.0 * zin_b1
    consts["WZI2_O"] = -zin_b02
    consts["WZI1_O"] = -zin_b1
    consts["WZO2_N"] = 2.0 * zout_b02
    consts["WZO1_N"] = 2.0 * zout_b1
    consts["WZO2_O"] = -zout_b02
    consts["WZO1_O"] = -zout_b1

    bgs = np.zeros((30, 64), np.float32)
    bgs[0, 0] = -st          # -st * b_in[0,0]
    bgs[29, 32 + 29] = st    # -st * b_out[29,29] = -st * (-1)
    consts["BGS"] = bgs

    return consts, float(tau), float(st)


# -------------------------------------------------------------- device side

def _sub_ap(ap, pairs):
    """Return a copy of `ap` with extra/replacement free-dim [step,count] axes."""
    out = ap.copy()
    out.ap = type(out.ap)(pairs)
    return out


def build_nc(consts, tau, n_iters=N_ITERS, peel=PEEL, unroll=UNROLL):
    nc = bacc.Bacc(None, target_bir_lowering=False, debug=False)

    dins = {
        name: nc.dram_tensor(name, list(arr.shape), F32, kind="ExternalInput")
        for name, arr in consts.items()
    }
    out_d = nc.dram_tensor("out", [XM, YM], F32, kind="ExternalOutput")

    peel = min(peel, n_iters)
    rem = n_iters - peel
    n_loops = rem // unroll
    tail = rem % unroll

    with tile.TileContext(nc) as tc:
        with (
            tc.tile_pool(name="sb", bufs=1) as sb,
            tc.tile_pool(name="ps", bufs=1, space="PSUM") as ps,
        ):
            X = [sb.tile([90, 100], F32, name=f"x{i}", tag=f"x{i}") for i in range(2)]
            U = sb.tile([30, 96], F32, name="u", tag="u")
            sbt = {
                name: sb.tile(list(arr.shape), F32, name=name.lower(), tag=name.lower())
                for name, arr in consts.items()
            }
            SQ = sb.tile([90, 96], F32, name="sq", tag="sq")
            RS = sb.tile([90, 1], F32, name="rs", tag="rs")
            NRM = sb.tile([90, 1], F32, name="nrm", tag="nrm")
            REC = sb.tile([90, 1], F32, name="rec", tag="rec")
            SVEC = sb.tile([90, 1], F32, name="svec", tag="svec")
            WS = sb.tile([90, 90], F32, name="ws", tag="ws")

            Tp = [ps.tile([90, 96], F32, name=f"t{i}", tag=f"t{i}") for i in range(2)]
            Zp = [ps.tile([30, 64], F32, name=f"z{i}", tag=f"z{i}") for i in range(2)]
            SS = ps.tile([90, 1], F32, name="ss", tag="ss")

            for name in consts:
                nc.sync.dma_start(sbt[name][:, :], dins[name][:, :])
            nc.vector.memset(X[0][:, :], 0.0)
            nc.vector.memset(X[1][:, :], 0.0)
            nc.vector.memset(U[:, :], 0.0)

            # in1 / out APs for the u-update (skip the 2-col pads)
            def u2ap():
                base = U[0:30, 32:62]
                return _sub_ap(base, [tuple(base.ap[0]), (32, 2), (1, 30)])

            def z2ap(Z):
                base = Z[0:30, 0:30]
                return _sub_ap(base, [tuple(base.ap[0]), (32, 2), (1, 30)])

            def emit_iter(t, exact_s):
                par = t % 2
                Xc, Xo = X[par], X[1 - par]
                T, Z = Tp[par], Zp[par]

                if exact_s:
                    nc.scalar.activation(
                        SQ[:, :], Xc[:, 2:98], ACTF.Square,
                        accum_out=RS[:, 0:1],
                    )
                    nc.tensor.matmul(
                        SS[:, 0:1], sbt["W_ONES90"][:, :], RS[:, 0:1],
                        start=True, stop=True,
                    )
                    nc.scalar.activation(NRM[:, 0:1], SS[:, 0:1], ACTF.Sqrt)
                    nc.vector.tensor_scalar(
                        REC[:, 0:1], NRM[:, 0:1], 1e-12, None, ALU.max)
                    nc.vector.reciprocal(REC[:, 0:1], REC[:, 0:1])
                    nc.vector.tensor_scalar(
                        SVEC[:, 0:1], REC[:, 0:1],
                        float(-tau * MU), 1.0, ALU.mult, ALU.add)
                    nc.vector.tensor_scalar(
                        WS[:, :], sbt["W_I90"][:, :], SVEC[:, 0:1], None,
                        ALU.mult)
                    wx = WS
                else:
                    wx = sbt["W_I90"]

                # ---- T = s*x + CONST - tau*(A^T y)   (y kept as U = tau*y)
                nc.tensor.matmul(T[:, 0:96], wx[:, :], Xc[:, 2:98],
                                 start=True, stop=False)
                nc.tensor.matmul(T[:, 0:96], sbt["W_I90"][:, :],
                                 sbt["CONST"][:, :], start=False, stop=False)
                if USE_BCAST_RHS:
                    uo = U[0:30, 64:96].unsqueeze(1).broadcast_to((30, 3, 32))
                    nc.tensor.matmul(T[:, 0:96], sbt["W_MOUT"][:, :], uo,
                                     start=False, stop=False)
                else:
                    for b in range(3):
                        nc.tensor.matmul(
                            T[:, 32 * b:32 * b + 32], sbt["W_MOUT"][:, :],
                            U[0:30, 64:96], start=False, stop=False)
                for b in range(3):
                    nc.tensor.matmul(
                        T[:, 32 * b:32 * b + 32], sbt["W_SHIFT"][:, :],
                        U[0:30, 31 + b:63 + b], start=False, stop=False)
                nc.tensor.matmul(T[:, 32:64], sbt["W_FIXO"][:, :],
                                 U[0:30, 64:96], start=False, stop=False)
                nc.tensor.matmul(T[:, 32:64], sbt["W_FIXI"][:, :],
                                 U[0:30, 32:64], start=False, stop=True)

                # ---- Z(old half): -st*A@x_old  plus the -st*b constant
                for b in range(3):
                    wz = sbt["WZI1_O"] if b == 1 else sbt["WZI2_O"]
                    nc.tensor.matmul(
                        Z[0:30, 0:32], wz[:, :],
                        Xc[:, 3 + 31 * b:35 + 31 * b],
                        start=(b == 0), stop=False)
                for b in range(3):
                    wz = sbt["WZO1_O"] if b == 1 else sbt["WZO2_O"]
                    nc.tensor.matmul(
                        Z[0:30, 32:64], wz[:, :],
                        Xc[:, 2 + 32 * b:34 + 32 * b],
                        start=False, stop=False)
                nc.tensor.matmul(Z[0:30, 0:64], sbt["W_I90"][0:30, 0:30],
                                 sbt["BGS"][:, :], start=False, stop=False)

                # ---- x_new = clip(T, 0, 1); INV lanes clip to exact 0
                nc.vector.tensor_scalar(
                    Xo[:, 2:98], T[:, 0:96], 0.0, 1.0, ALU.max, ALU.min)

                # ---- Z(new half): +2*st*A@x_new
                for b in range(3):
                    wz = sbt["WZI1_N"] if b == 1 else sbt["WZI2_N"]
                    nc.tensor.matmul(
                        Z[0:30, 0:32], wz[:, :],
                        Xo[:, 3 + 31 * b:35 + 31 * b],
                        start=False, stop=False)
                for b in range(3):
                    wz = sbt["WZO1_N"] if b == 1 else sbt["WZO2_N"]
                    nc.tensor.matmul(
                        Z[0:30, 32:64], wz[:, :],
                        Xo[:, 2 + 32 * b:34 + 32 * b],
                        start=False, stop=(b == 2))

                # ---- U += Z
                nc.vector.tensor_tensor(u2ap(), u2ap(), z2ap(Z), ALU.add)

            t = 0
            for _ in range(peel):
                emit_iter(t, True)
                t += 1
            if n_loops:
                with tc.For_i(0, n_loops) as _i:
                    for k in range(unroll):
                        emit_iter(t + k, False)
                t += n_loops * unroll
            for _ in range(tail):
                emit_iter(t, False)
                t += 1

            xfin = X[n_iters % 2]
            nc.sync.dma_start(out_d[:, :], xfin[30:60, 34:64])

    nc.compile()
    return nc


# ------------------------------------------------------------------- entry

def _run(weights, A, b, trace=False, **spmd_kwargs):
    consts, tau, _st = _host_prep(weights, A)
    nc = build_nc(consts, tau)
    in_map = {k: np.ascontiguousarray(v) for k, v in consts.items()}
    res = run_bass_kernel_spmd(
        nc, [dict(in_map) for _ in range(N_CORES)], list(range(N_CORES)),
        trace=trace, **spmd_kwargs)
    out = np.asarray(res.results[0]["out"], np.float32)
    return out, res


def kernel(weights, A, b):
    out, _res = _run(weights, A, b)
    return out


# revision 7
# speedup vs baseline: 1.3050x; 1.3050x over previous
"""Trainium2 Bass kernel for nn_CvxDifflayer (30x30 grid shortest-path LP via PDHG).

Strategy
--------
The LP matrix A is the (negated) oriented incidence matrix of a 30x30 grid with
8-neighbor connectivity: 900 "internal" edges (in->out of each cell, carrying
the weights as costs) plus 6844 neighbor edges.  A@x and A.T@y are therefore
8-neighbor *stencils* on the grid, not dense matmuls.

Device layout: edges live in a 90-partition x 96-column fp32 tile.  Partition
30*a+i holds grid row i of direction-block a (a = p+1, p in {-1,0,1}); column
32*b+j holds grid column j of direction-block b (b = q+1).  Chunk (a,b)=(1,1)
is the internal edge; the other 8 chunks are the neighbor directions.  All of
the PDHG linear algebra (A.T y stencil, A x_bar stencil with x_bar = 2*x_new -
x folded in, constants, and masking via a -2^100 additive "INV" term that the
[0,1] clip turns into exact zeros) runs as small PSUM-accumulating TensorE
matmuls; each iteration needs only two VectorE ops (the clip and the y-state
update).

The step size tau = 0.95/||A||_2 is computed on the host with the same fp32
power iteration the reference uses (the trajectory is insensitive to ulp-level
tau perturbations; validated numerically).  The ||x|| ridge term (MU = 1e-8)
affects fp32 results only while ||x|| is tiny, so the exact per-iteration norm
chain runs for the first PEEL iterations and s = 1 - tau*MU/||x|| is exactly
1.0f afterwards (validated bit-identical).

All 8 NeuronCores run the identical single-core program (the LP is one small
sequential solve; per the sharding hint replication beats tensor parallelism);
core 0's output is returned.
"""

import sys

import numpy as np

sys.path.insert(0, "/opt/trn_rl_repo")

import concourse.bacc as bacc
import concourse.mybir as mybir
import concourse.tile as tile
from concourse.bass_utils import run_bass_kernel_spmd

F32 = mybir.dt.float32
ALU = mybir.AluOpType
ACTF = mybir.ActivationFunctionType

XM = YM = 30
MU = 1e-8
N_ITERS = 1500
PEEL = 16            # iterations with the exact ||x|| ridge term
UNROLL = 28          # loop body unroll (even, preserves buffer parity)
BIG = float(2.0 ** 100)
N_CORES = 8
USE_BCAST_RHS = True  # stride-0 rhs AP for the u_out broadcast matmul


# ---------------------------------------------------------------- host side

def _host_prep(weights, A):
    """Decode the grid structure from A, run the fp32 power iteration for tau,
    and build every constant/weight tile the device program needs."""
    A = np.asarray(A, np.float32)
    w = np.asarray(weights, np.float32)
    V = A.shape[1]

    tails = np.argmax(A, 0)
    heads = np.argmin(A, 0)
    mask = np.zeros((3, 3, XM, YM), np.float32)
    for e in range(V):
        t, h = int(tails[e]), int(heads[e])
        i, j = divmod(t // 2, YM)
        if t % 2 == 0:
            mask[1, 1, i, j] = 1.0
        else:
            i2, j2 = divmod(h // 2, YM)
            mask[i2 - i + 1, j2 - j + 1, i, j] = 1.0

    # fp32 power iteration (same op structure as the reference)
    v = np.full(V, 1.0 / np.sqrt(np.float32(V)), np.float32)
    for _ in range(30):
        wv = A.T @ (A @ v)
        v = wv / np.float32(np.linalg.norm(wv))
    L = np.float32(np.linalg.norm(A @ v))
    tau = np.float32(0.95) / L
    st = np.float32(tau * tau)

    consts = {}

    # CONST tile: -BIG at invalid/pad lanes (clipped to exact 0), -tau*c at ch4
    const = np.zeros((90, 96), np.float32)
    for a in range(3):
        for b in range(3):
            blk = const[30 * a:30 * a + 30, 32 * b:32 * b + 32]
            blk[:, 30:] = -BIG
            blk[:, :30] = np.where(mask[a, b] > 0, 0.0, -BIG)
    const[30:60, 32:62] += -tau * w
    consts["CONST"] = const

    consts["W_I90"] = np.eye(90, dtype=np.float32)
    consts["W_ONES90"] = np.ones((90, 90), np.float32)

    # -u_out broadcast into all 9 chunks
    w_mout = np.zeros((30, 90), np.float32)
    for a in range(3):
        for i in range(30):
            w_mout[i, 30 * a + i] = -1.0
    consts["W_MOUT"] = w_mout

    # +u_in[i+p, j+q] shifts (partition direction handled by W, q by rhs offset)
    w_shift = np.zeros((30, 90), np.float32)
    for a in range(3):
        p = a - 1
        for i in range(30):
            if 0 <= i + p < 30:
                w_shift[i + p, 30 * a + i] = 1.0
    consts["W_SHIFT"] = w_shift

    # ch4 correction: uniform chunks gave -u_out + u_in; target +u_out - u_in
    w_fixo = np.zeros((30, 90), np.float32)
    w_fixi = np.zeros((30, 90), np.float32)
    for i in range(30):
        w_fixo[i, 30 + i] = 2.0
        w_fixi[i, 30 + i] = -2.0
    consts["W_FIXO"] = w_fixo
    consts["W_FIXI"] = w_fixi

    # Z weights: Z accumulates st*A@(2*x_new - x_old) - st*b
    zin_b02 = np.zeros((90, 30), np.float32)
    for a in range(3):
        p = a - 1
        for i2 in range(30):
            i = i2 - p
            if 0 <= i < 30:
                zin_b02[30 * a + i, i2] = -st
    zin_b1 = zin_b02.copy()
    zin_b1[30:60, :] *= -1.0

    zout_b02 = np.zeros((90, 30), np.float32)
    for a in range(3):
        for i in range(30):
            zout_b02[30 * a + i, i] = st
    zout_b1 = zout_b02.copy()
    zout_b1[30:60, :] *= -1.0

    consts["WZI2"] = zin_b02
    consts["WZI1"] = zin_b1
    consts["WZO2"] = zout_b02
    consts["WZO1"] = zout_b1

    bgs = np.zeros((30, 64), np.float32)
    bgs[0, 0] = -st          # -st * b_in[0,0]
    bgs[29, 32 + 29] = st    # -st * b_out[29,29] = -st * (-1)
    consts["BGS"] = bgs

    return consts, float(tau), float(st)


# -------------------------------------------------------------- device side

def _sub_ap(ap, pairs):
    """Return a copy of `ap` with extra/replacement free-dim [step,count] axes."""
    out = ap.copy()
    out.ap = type(out.ap)(pairs)
    return out


def build_nc(consts, tau, n_iters=N_ITERS, peel=PEEL, unroll=UNROLL,
             f32r=frozenset()):
    nc = bacc.Bacc(None, target_bir_lowering=False, debug=False)

    dins = {
        name: nc.dram_tensor(name, list(arr.shape), F32, kind="ExternalInput")
        for name, arr in consts.items()
    }
    out_d = nc.dram_tensor("out", [XM, YM], F32, kind="ExternalOutput")

    peel = min(peel, n_iters)
    rem = n_iters - peel
    n_loops = rem // unroll
    tail = rem % unroll

    with tile.TileContext(nc) as tc:
        with (
            tc.tile_pool(name="sb", bufs=1) as sb,
            tc.tile_pool(name="ps", bufs=1, space="PSUM") as ps,
        ):
            X = [sb.tile([90, 100], F32, name=f"x{i}", tag=f"x{i}") for i in range(2)]
            U = sb.tile([30, 96], F32, name="u", tag="u")
            sbt = {
                name: sb.tile(list(arr.shape), F32, name=name.lower(), tag=name.lower())
                for name, arr in consts.items()
            }
            SQ = sb.tile([90, 96], F32, name="sq", tag="sq")
            RS = sb.tile([90, 1], F32, name="rs", tag="rs")
            NRM = sb.tile([90, 1], F32, name="nrm", tag="nrm")
            REC = sb.tile([90, 1], F32, name="rec", tag="rec")
            SVEC = sb.tile([90, 1], F32, name="svec", tag="svec")
            XB = sb.tile([90, 100], F32, name="xb", tag="xb")

            Tp = [ps.tile([90, 96], F32, name=f"t{i}", tag=f"t{i}") for i in range(2)]
            Zp = [ps.tile([30, 64], F32, name=f"z{i}", tag=f"z{i}") for i in range(2)]
            SS = ps.tile([90, 1], F32, name="ss", tag="ss")

            for name in consts:
                nc.sync.dma_start(sbt[name][:, :], dins[name][:, :])
            nc.vector.memset(X[0][:, :], 0.0)
            nc.vector.memset(X[1][:, :], 0.0)
            nc.vector.memset(U[:, :], 0.0)
            nc.vector.memset(XB[:, :], 0.0)

            def mmcast(ap, key):
                return ap.bitcast(mybir.dt.float32r) if key in f32r else ap

            def mm(out, lhsT, rhs, key, **kw):
                nc.tensor.matmul(out, mmcast(lhsT, key), mmcast(rhs, key),
                                 skip_group_check=True, **kw)

            # One-time warm-up matmuls: set has_written on every T-bank
            # element so later start=False matmuls accumulate onto the
            # VectorE-written base instead of overwriting it.
            for i in range(2):
                nc.tensor.matmul(Tp[i][:, 0:96], sbt["W_I90"][:, :],
                                 sbt["CONST"][:, :], start=True, stop=True)

            def shift_rhs():
                base = U[0:30, 31:63]
                return _sub_ap(base, [tuple(base.ap[0]), (1, 3), (1, 32)])

            def emit_iter(t, exact_s):
                par = t % 2
                Xc, Xo = X[par], X[1 - par]
                T, Z = Tp[par], Zp[par]

                # ---- T base (VectorE writes PSUM): CONST + s*x
                if exact_s:
                    nc.scalar.activation(
                        SQ[:, :], Xc[:, 2:98], ACTF.Square,
                        accum_out=RS[:, 0:1],
                    )
                    nc.tensor.matmul(
                        SS[:, 0:1], sbt["W_ONES90"][:, :], RS[:, 0:1],
                        start=True, stop=True,
                    )
                    nc.scalar.activation(NRM[:, 0:1], SS[:, 0:1], ACTF.Sqrt)
                    nc.vector.tensor_scalar(
                        REC[:, 0:1], NRM[:, 0:1], 1e-12, None, ALU.max)
                    nc.vector.reciprocal(REC[:, 0:1], REC[:, 0:1])
                    nc.vector.tensor_scalar(
                        SVEC[:, 0:1], REC[:, 0:1],
                        float(-tau * MU), 1.0, ALU.mult, ALU.add)
                    nc.vector.scalar_tensor_tensor(
                        T[:, 0:96], Xc[:, 2:98], SVEC[:, 0:1],
                        sbt["CONST"][:, :], ALU.mult, ALU.add)
                else:
                    nc.vector.tensor_tensor(
                        T[:, 0:96], sbt["CONST"][:, :], Xc[:, 2:98], ALU.add)

                # ---- Z: the -st*b constant first (resets the bank)
                mm(Z[0:30, 0:64], sbt["W_I90"][0:30, 0:30], sbt["BGS"][:, :],
                   "b", start=True, stop=False)

                # ---- T += -tau*(A^T y): u_in shifts (one matmul), ch4 fixes,
                #      -u_out broadcast   (PE accumulates onto the V base)
                mm(T[:, 0:96], sbt["W_SHIFT"][:, :], shift_rhs(), "shift",
                   start=False, stop=False)
                mm(T[:, 32:64], sbt["W_FIXI"][:, :], U[0:30, 32:64], "fix",
                   start=False, stop=False)
                uo = U[0:30, 64:96].unsqueeze(1).broadcast_to((30, 3, 32))
                mm(T[:, 0:96], sbt["W_MOUT"][:, :], uo, "uout",
                   start=False, stop=False)
                mm(T[:, 32:64], sbt["W_FIXO"][:, :], U[0:30, 64:96], "fix",
                   start=False, stop=True)

                # ---- x_new = clip(T, 0, 1); INV lanes clip to exact 0
                nc.vector.tensor_scalar(
                    Xo[:, 2:98], T[:, 0:96], 0.0, 1.0, ALU.max, ALU.min)

                # ---- x_bar = 2*x_new - x  (VectorE)
                nc.vector.tensor_scalar(
                    XB[:, 2:98], Xo[:, 2:98], 2.0, None, ALU.mult)
                nc.vector.tensor_tensor(
                    XB[:, 2:98], XB[:, 2:98], Xc[:, 2:98], ALU.subtract)

                # ---- Z += st*A@x_bar
                for b in range(3):
                    wz = sbt["WZI1"] if b == 1 else sbt["WZI2"]
                    mm(Z[0:30, 0:32], wz[:, :], XB[:, 3 + 31 * b:35 + 31 * b],
                       "z", start=False, stop=False)
                for b in range(3):
                    wz = sbt["WZO1"] if b == 1 else sbt["WZO2"]
                    mm(Z[0:30, 32:64], wz[:, :], XB[:, 2 + 32 * b:34 + 32 * b],
                       "z", start=False, stop=(b == 2))

                # ---- U += Z  (split so u_in consumers can start earlier)
                nc.vector.tensor_tensor(
                    U[0:30, 32:62], U[0:30, 32:62], Z[0:30, 0:30], ALU.add)
                nc.vector.tensor_tensor(
                    U[0:30, 64:94], U[0:30, 64:94], Z[0:30, 32:62], ALU.add)

            t = 0
            for _ in range(peel):
                emit_iter(t, True)
                t += 1
            if n_loops:
                with tc.For_i(0, n_loops) as _i:
                    for k in range(unroll):
                        emit_iter(t + k, False)
                t += n_loops * unroll
            for _ in range(tail):
                emit_iter(t, False)
                t += 1

            xfin = X[n_iters % 2]
            nc.sync.dma_start(out_d[:, :], xfin[30:60, 34:64])

    nc.compile()
    return nc


# ------------------------------------------------------------------- entry

def _run(weights, A, b, trace=False, f32r=frozenset(), **spmd_kwargs):
    consts, tau, _st = _host_prep(weights, A)
    nc = build_nc(consts, tau, f32r=f32r)
    in_map = {k: np.ascontiguousarray(v) for k, v in consts.items()}
    res = run_bass_kernel_spmd(
        nc, [dict(in_map) for _ in range(N_CORES)], list(range(N_CORES)),
        trace=trace, **spmd_kwargs)
    out = np.asarray(res.results[0]["out"], np.float32)
    return out, res


def kernel(weights, A, b):
    out, _res = _run(weights, A, b)
    return out
